# revision 18
# baseline (speedup 1.0000x reference)
"""Trainium2 Bass kernel for AttentionStem (sparse local 4x4-window attention).

Contract: kernel(**inputs) takes the FULL unsharded inputs (numpy, keyed as in
setup_inputs()) and returns the FULL output [4, 64, 128, 128] float32.

Algorithm (per output pixel (b, h, w), per channel o):
    q  = query_w @ x                    (1x1 conv)
    kc = key_w @ xpad                   (1x1 conv on padded grid)
    vs_k = W_k @ xpad,  W_k = sum_m softmax_m(emb)[m,k] * value_w[m]
    attn_k = softmax_k(q * kc[pix + off_k])        (16 window positions)
    out = sum_k attn_k * vs_k[pix + off_k]

Sharding: 8 cores = 4 batches x 2 H-halves (64 output rows each, 4-row halo).

v3 design (~170us vs the ~214us previous kernel under identical harness
conditions; all engines 60-82% busy):
  - Block-stacked convs: the two 32-row blocks live on partitions 0-63 /
    64-127; one contraction-6 matmul (stationary [6,128] block-diagonal
    weights, moving [6,N] doubled input slab) computes both, halving the
    matmul count vs per-block convs.
  - The softmax denominator s = sum_k e_k is accumulated on the
    TensorEngine via identity-matmul PSUM accumulation (f32, exact),
    taking that work off the saturated VectorE. (Accumulating num there
    too overloads the cold-clocked PE - knob K_NACC_PE, off.)
  - Window positions are processed in column-quads (fixed j, i=0..3): one
    flat DVE op each for logit/exp/vmul and two for num-acc per quad,
    minimizing per-op overheads and 132-stride resyncs.
  - Software-pipelined: each quad's value convs are emitted one quad
    ahead (the PE queue is FIFO - without the prefetch it stalls behind
    s-acc matmuls that wait on exp), q/kcv convs one section ahead,
    ep/vs2 pools triple-buffered, finals split into column halves.
  - All elementwise planes are flat [128, n*128] (compact, no pad
    columns); only the kcv operand of the logit multiply keeps the
    132-padded layout (plus a 1-shifted twin so both column parities stay
    4B-aligned for the DVE 2x tensor_tensor mode).
  - PSUM->SBUF vs evacuation is mostly on ScalarE (knob splits a few ops
    to DVE); sections are 8 rows so s_ps + conv PSUM fit in 8 banks.
Engine-balance at the end: ScalarE ~82% (exp + evacuation), VectorE ~79%
(logit/vmul/num-acc/finals), PE ~63% (convs + s-acc, mostly at the cold
1.2GHz clock - the 1-2% array utilization never trips the HAM warm-up).
"""

import os
import sys

import numpy as np

sys.path.insert(0, "/opt/trn_rl_repo")

# Problem constants (hardcoded; kernel.py must be self-contained).
B, IC, OC, H, W = 4, 3, 64, 128, 128
KS, PAD, M = 4, 2, 4
NCORES = 8

W132 = W + 2 * PAD  # padded width = 132
SH_OUT_ROWS = 64  # output rows per core
BLK = 32  # output rows per partition-block
SLAB_ROWS = BLK + KS - 1  # padded input rows needed per block = 35
SLAB_FREE = SLAB_ROWS * W132 + 16  # doubled slab free size (+shift pad)
KCV_FREE = SLAB_ROWS * W132  # kcv plane size (padded layout) = 4620
KCV_PAD = 16
SECR = int(os.environ.get("K_SECR", "8"))  # section rows (per block)
NSEC = BLK // SECR  # sections per core
SF = SECR * W  # flat plane elems per section
NQ = 4  # quads (one per column shift j)

# Config knobs.
CFG = {
    # number of the 64 per-tile vs evacuations assigned to DVE (rest ScalarE)
    "evac_dve": int(os.environ.get("K_EVAC_DVE", "4")),
    # how many of the 4 e-slots per quad accumulate s on the TensorEngine
    # (the rest ride a DVE bf16 2-slot accumulator)
    "sacc_pe_slots": int(os.environ.get("K_SACC_PE", "4")),
    # how many of the 4 vmul slots per quad accumulate num on the
    # TensorEngine (identity matmuls into n_ps); the rest ride the DVE
    # bf16 accumulator. 4 needs SECR<=8 for PSUM banks.
    "nacc_pe": int(os.environ.get("K_NACC_PE", "0")),
    # split exp/vmul into this many pieces (shortens logit->exp->vmul chain)
    "halves": int(os.environ.get("K_HALVES", "1")),
    # reciprocal reads s_ps straight from PSUM (drops the s_f ScalarE copy)
    "rinv_psum": os.environ.get("K_RINV_PSUM", "0") == "1",
    # kcv PSUM->SBUF evacuation on DVE instead of ScalarE (S is the
    # bottleneck; V runs it at 1x but has headroom)
    "kcv_v": os.environ.get("K_KCV_V", "0") == "1",
    # final num accumulator tile in f32 (kills the implicit CASTs before
    # the out = num * rinv multiply; the pair-add drops to 1x)
    "numf32": os.environ.get("K_NUMF32", "0") == "1",
    # one-slot num accumulator: 4 smaller DVE adds per quad instead of 2
    # wide ones, but the section-final pair-add disappears entirely
    "nacc1": os.environ.get("K_NACC1", "0") == "1",
    # section finals on GpSimd: 1 = pair-add, 2 = pair-add + out-mul
    "fin_pool": int(os.environ.get("K_FIN_POOL", "0")),
    # one-quad software pipeline on the DVE side: vmul/accs of quad t run
    # while ScalarE exps quad t+1, so V never stalls on the exp of its own
    # quad (the V queue is FIFO; without this, vmul(t) blocks logit(t+1))
    "pipe": os.environ.get("K_PIPE", "0") == "1",
}

_CACHE = {}
LAST_RESULT = None  # BassKernelResults of the most recent run (for test.py)


def _emit(nc, tc, aps, cfg):
    """Emit the per-core program.

    aps: slab [6, SLAB_FREE] bf16, wts [6, 18*128] bf16, ident [128,128]
    bf16, out [64, 64, 128] f32.
    wts slot s in {0=q, 1=k, 2+k}: [0:3, 128s:128s+64] = W.T (block0),
    [3:6, 128s+64:128s+128] = W.T (block1)."""
    from contextlib import ExitStack

    import concourse.bass as bass
    from concourse import mybir

    f32 = mybir.dt.float32
    bf16 = mybir.dt.bfloat16
    EXP = mybir.ActivationFunctionType.Exp
    spe = cfg["sacc_pe_slots"]
    npe = cfg["nacc_pe"]
    assert not (npe and spe and SECR > 8), "PSUM: both accs on PE need SECR<=8"
    assert not (cfg["pipe"] and npe), "PSUM: pipe doubles spsum; no room for nacc_pe"

    # quad column order: even parities first so the kcv1 shifted-twin DMA
    # has slack before the first odd-j logit needs it
    QORDER = (0, 2, 1, 3)

    with ExitStack() as ctx:
        const = ctx.enter_context(tc.tile_pool(name="const", bufs=1))
        qkp = ctx.enter_context(tc.tile_pool(name="qk", bufs=1))
        psum = ctx.enter_context(tc.tile_pool(name="psum", bufs=2, space="PSUM"))
        spsum = ctx.enter_context(
            tc.tile_pool(name="spsum", bufs=2 if cfg["pipe"] else 1,
                         space="PSUM")
        )
        vsp = ctx.enter_context(tc.tile_pool(name="vs", bufs=int(os.environ.get("K_VSB","3"))))
        epp = ctx.enter_context(tc.tile_pool(name="ep", bufs=int(os.environ.get("K_EPB","3"))))
        accp = ctx.enter_context(
            tc.tile_pool(name="acc", bufs=2 if cfg["pipe"] else 1)
        )
        finp = ctx.enter_context(tc.tile_pool(name="fin", bufs=1))
        outp = ctx.enter_context(tc.tile_pool(name="out", bufs=2))

        slab = const.tile([6, SLAB_FREE], bf16, tag="slab", name="slab")
        wts = const.tile([6, 18 * 128], bf16, tag="wts", name="wts")
        ident = const.tile([128, 128], bf16, tag="ident", name="ident")

        # input DMAs: weights + identity + the slab piece the first section
        # needs, then the rest — on separate queues so they land in parallel.
        # The q/kcv weight slots and the first 9 slab rows go first so the
        # very first conv chunk isn't gated on the whole transfer.
        P0 = 9 * W132
        P1 = 20 * W132
        nc.sync.dma_start(wts[:, 0:256], aps["wts"][:, 0:256])
        nc.gpsimd.dma_start(slab[:, 0:P0], aps["slab"][:, 0:P0])
        nc.sync.dma_start(wts[:, 256:], aps["wts"][:, 256:])
        nc.gpsimd.dma_start(slab[:, P0:P1], aps["slab"][:, P0:P1])
        nc.sync.dma_start(ident[:], aps["ident"][:])
        nc.scalar.dma_start(slab[:, P1:SLAB_FREE], aps["slab"][:, P1:SLAB_FREE])

        # persistent q (flat compact) and kcv (padded layout) tiles
        q = qkp.tile([128, NSEC * SF], bf16, tag="q", name="q")
        kcv0 = qkp.tile([128, KCV_FREE + KCV_PAD], bf16, tag="kcv0", name="kcv0")
        kcv1 = qkp.tile([128, KCV_FREE + KCV_PAD], bf16, tag="kcv1", name="kcv1")

        evac_ctr = [0]

        def evac(dst, src):
            """PSUM->SBUF evacuation, engine chosen round-robin by knob."""
            n = evac_ctr[0]
            evac_ctr[0] += 1
            # spread the DVE share evenly through the run
            if (n * cfg["evac_dve"]) % 64 < cfg["evac_dve"]:
                nc.vector.tensor_copy(dst, src)
            else:
                nc.scalar.copy(dst, src)

        def conv_compact(wslot, dst, dst_off, base, nrows):
            """dst[:, dst_off + r*128 + w] = conv(slot) at slab offset
            base + r*132 + w, for r in [0, nrows). Chunks of 4 rows = 512
            cols. dst is a flat bf16 SBUF tile; evac engine via knob."""
            w_l = wts[:, 128 * wslot : 128 * (wslot + 1)]
            r = 0
            while r < nrows:
                sub = min(8, nrows - r)  # one pt tile = 2 matmuls = 8 rows
                pt = psum.tile([128, 1024], f32, tag="pt", name="pt")
                rr = 0
                while rr < sub:
                    s4 = min(4, sub - rr)
                    lo = base + (r + rr) * W132
                    rhs = slab[:, lo : lo + s4 * W132]
                    rhs3 = rhs.rearrange("c (r w) -> c r w", w=W132)[:, :, 0:W]
                    nc.tensor.matmul(
                        pt[:, rr * W : (rr + s4) * W], w_l, rhs3
                    )
                    rr += s4
                evac(
                    dst[:, dst_off + r * W : dst_off + (r + sub) * W],
                    pt[:, : sub * W],
                )
                r += sub

        def conv_padded(wslot, dst, lo, hi, ev):
            """dst[:, f] = conv(slot) at slab offset f for f in [lo, hi):
            contiguous padded-layout conv (for kcv)."""
            w_l = wts[:, 128 * wslot : 128 * (wslot + 1)]
            off = lo
            while off < hi:
                n = min(1024, hi - off)
                pt = psum.tile([128, 1024], f32, tag="pt", name="pt")
                coff = 0
                while coff < n:
                    cn = min(512, n - coff)
                    nc.tensor.matmul(
                        pt[:, coff : coff + cn],
                        w_l,
                        slab[:, off + coff : off + coff + cn],
                    )
                    coff += cn
                ev(dst[:, off : off + n], pt[:, :n])
                off += n

        def qk_phase(sec):
            """q (compact) + kcv (padded) convs for one section, then the
            kcv1 shifted twin via DMA."""
            # kcv rows [sec*16, sec*16+19) except the last section stops at 35
            klo_r = sec * SECR if sec == 0 else sec * SECR + KS - 1
            khi_r = sec * SECR + SECR + KS - 1
            klo, khi = klo_r * W132, khi_r * W132
            # during the ramp the DVE is idle; evacuate there to unload ScalarE
            kev = (
                nc.vector.tensor_copy
                if (sec == 0 or cfg["kcv_v"])
                else nc.scalar.copy
            )
            conv_padded(1, kcv0, klo, khi, kev)
            conv_compact(0, q, sec * SF, (sec * SECR + PAD) * W132 + PAD, SECR)
            last = sec == NSEC - 1
            if last:
                nc.vector.memset(kcv0[:, KCV_FREE:], 0.0)
            # kcv1[f] = kcv0[f+1], in partition quarters on the DMA engines
            lo = klo - 1 if klo > 0 else 0
            hi = (KCV_FREE + KCV_PAD - 8) if last else khi - 1
            for p0 in range(0, 128, 32):
                nc.sync.dma_start(
                    kcv1[p0 : p0 + 32, lo:hi], kcv0[p0 : p0 + 32, lo + 1 : hi + 1]
                )
            if last:
                nc.vector.memset(kcv1[:, KCV_FREE:], 0.0)

        def logit_quad(j, sec, ep):
            """ep[:, 0:4*SF] = q * kcv shifted, for the 4 slots i=0..3 of
            column shift j (window position k = 4i + j). One DVE op: the
            kcv operand gets a [132, 4] outer slot dim, q a [0, 4]
            broadcast dim, the dst a [SF, 4] flat slot dim."""
            if j % 2 == 1:
                ksrc, koff = kcv1, j - 1
            else:
                ksrc, koff = kcv0, j
            base = sec * SECR * W132 + koff
            vk = ksrc[:, base : base + SECR * W132]
            vk3 = vk.rearrange("p (r w) -> p r w", w=W132)[:, :, 0:W]
            kk = bass.AP(
                vk3.tensor, vk3.offset, [list(vk3.ap)[0], [W132, 4], *list(vk3.ap)[1:]]
            )
            vq = q[:, sec * SF : (sec + 1) * SF]
            qq = bass.AP(
                vq.tensor, vq.offset, [list(vq.ap)[0], [0, 4], *list(vq.ap)[1:]]
            )
            dst = ep[:, 0 : 4 * SF].rearrange("p (s n) -> p s n", s=4)
            nc.vector.tensor_mul(dst, qq, kk)

        # ---- main loop: software-pipelined over 8 quad-secs ----
        NT = NSEC * NQ

        def convs(t):
            """value convs for quad-sec t -> a fresh vs2 tile."""
            sec, qi = divmod(t, NQ)
            j = QORDER[qi]
            vs2 = vsp.tile([128, 4 * SF], bf16, tag="vs2", name="vs2")
            for i in range(4):
                conv_compact(
                    2 + 4 * i + j, vs2, i * SF,
                    (sec * SECR + i) * W132 + j, SECR,
                )
            return vs2

        qk_phase(0)
        vs2 = convs(0)
        NCH = SF // 512  # 512-col chunks per plane-section
        nh = cfg["halves"]
        HF = 4 * SF // nh
        acc = {}  # accumulator tiles of the tail stream's current section

        def finals(sec, s_ps, n_ps, accn, acce):
            # ---- section finals, in column halves so the store DMA and
            # the last quad's accumulate pipeline ----
            HS = SF // 2
            for h in range(2):
                lo, hi = h * HS, (h + 1) * HS
                if npe:
                    num = n_ps[:, lo:hi]
                elif cfg["nacc1"]:
                    num = accn[:, lo:hi]  # already fully accumulated
                else:
                    num = finp.tile([128, HS], f32 if cfg["numf32"] else bf16,
                                    tag="num", name="num")
                    fadd = (nc.gpsimd.tensor_add if cfg["fin_pool"] >= 1
                            else nc.vector.tensor_add)
                    fadd(num[:], accn[:, lo:hi], accn[:, SF + lo : SF + hi])
                    num = num[:]
                s_f = finp.tile([128, HS], f32, tag="sf", name="sf")
                s_src = None
                if spe == 4:
                    if cfg["rinv_psum"]:
                        s_src = s_ps[:, lo:hi]
                    else:
                        nc.scalar.copy(s_f[:], s_ps[:, lo:hi])
                elif spe > 0:
                    s_bf = finp.tile([128, HS], bf16, tag="sbf", name="sbf")
                    nc.vector.tensor_add(
                        s_bf[:], acce[:, lo:hi], acce[:, SF + lo : SF + hi]
                    )
                    nc.vector.tensor_add(s_f[:], s_ps[:, lo:hi], s_bf[:])
                else:
                    s2 = finp.tile([128, 2 * HS], bf16, tag="s2", name="s2")
                    nc.vector.tensor_add(
                        s2[:].rearrange("p (g n) -> p g n", g=2),
                        acce[:].rearrange("p (g n) -> p g n", g=4)[:, 0:2, lo:hi],
                        acce[:].rearrange("p (g n) -> p g n", g=4)[:, 2:4, lo:hi],
                    )
                    nc.vector.tensor_add(s_f[:], s2[:, 0:HS], s2[:, HS : 2 * HS])
                rinv = finp.tile([128, HS], f32, tag="rinv", name="rinv")
                nc.vector.reciprocal_approx_fast(
                    rinv[:], s_src if s_src is not None else s_f[:]
                )
                o_t = outp.tile([128, HS], f32, tag="o", name="o")
                omul = (nc.gpsimd.tensor_mul if cfg["fin_pool"] >= 2
                        else nc.vector.tensor_mul)
                omul(o_t[:], num, rinv[:])
                o_v = o_t[:].rearrange("p (r w) -> p r w", w=W)
                r0 = sec * SECR + h * (SECR // 2)
                # spread the store DMAs across queues so the tail drains fast
                qs = (nc.sync, nc.gpsimd, nc.scalar, nc.sync)
                for b in (0, 1):
                    qs[2 * h + b if sec == NSEC - 1 else b].dma_start(
                        aps["out"][:, b * BLK + r0 : b * BLK + r0 + SECR // 2, :],
                        o_v[64 * b : 64 * (b + 1)],
                    )

        def tail(ep, vs2_t, sec, qi):
            """vmul + accumulations (+ section finals) for one quad."""
            if qi == 0:
                acc.clear()
                if spe:
                    acc["s_ps"] = spsum.tile([128, SF], f32, tag="sacc",
                                             name="sacc")
                if npe:
                    acc["n_ps"] = spsum.tile([128, SF], f32, tag="nacc",
                                             name="nacc")
                elif cfg["nacc1"]:
                    acc["accn"] = accp.tile([128, SF], bf16, tag="accn",
                                            name="accn")
                else:
                    acc["accn"] = accp.tile([128, 2 * SF], bf16, tag="accn",
                                            name="accn")
                if spe < 4:
                    acc["acce"] = accp.tile([128, (4 - spe) * SF], bf16,
                                            tag="acce", name="acce")
            s_ps, n_ps = acc.get("s_ps"), acc.get("n_ps")
            accn, acce = acc.get("accn"), acc.get("acce")
            for h in range(nh):
                nc.vector.tensor_mul(
                    ep[:, 4 * SF + h * HF : 4 * SF + (h + 1) * HF],
                    ep[:, h * HF : (h + 1) * HF],
                    vs2_t[:, h * HF : (h + 1) * HF],
                )
            # s accumulation: slots [0, spe) on the PE (identity matmuls with
            # PSUM accumulate), slots [spe, 4) on a DVE bf16 accumulator
            for i in range(spe):
                for c in range(NCH):
                    nc.tensor.matmul(
                        s_ps[:, c * 512 : (c + 1) * 512],
                        ident[:],
                        ep[:, i * SF + c * 512 : i * SF + (c + 1) * 512],
                        start=(qi == 0 and i == 0),
                        stop=(qi == NQ - 1 and i == spe - 1),
                    )
            if spe < 4:
                lo, hi = spe * SF, 4 * SF
                if qi == 0:
                    nc.vector.tensor_copy(acce[:, 0 : hi - lo], ep[:, lo:hi])
                else:
                    nc.vector.tensor_add(
                        acce[:, 0 : hi - lo], acce[:, 0 : hi - lo], ep[:, lo:hi]
                    )
            # num accumulation: PE identity matmuls into PSUM, a single DVE
            # bf16 slot, or two DVE bf16 slots [n0|n1] (+)= [p0|p1], [p2|p3]
            if npe:
                for i in range(4):
                    for c in range(NCH):
                        nc.tensor.matmul(
                            n_ps[:, c * 512 : (c + 1) * 512],
                            ident[:],
                            ep[:, (4 + i) * SF + c * 512 :
                               (4 + i) * SF + (c + 1) * 512],
                            start=(qi == 0 and i == 0),
                            stop=(qi == NQ - 1 and i == 3),
                        )
            elif cfg["nacc1"]:
                for i in range(4):
                    src = ep[:, (4 + i) * SF : (5 + i) * SF]
                    if qi == 0 and i == 0:
                        nc.vector.tensor_copy(accn[:], src)
                    else:
                        nc.vector.tensor_add(accn[:], accn[:], src)
            else:
                if qi == 0:
                    nc.vector.tensor_copy(accn[:], ep[:, 4 * SF : 6 * SF])
                else:
                    nc.vector.tensor_add(accn[:], accn[:],
                                         ep[:, 4 * SF : 6 * SF])
                eng2 = (nc.gpsimd if os.environ.get("K_ACC_POOL", "0") == "1"
                        else nc.vector)
                eng2.tensor_add(accn[:], accn[:], ep[:, 6 * SF : 8 * SF])
            if qi == NQ - 1:
                finals(sec, s_ps, n_ps, accn, acce)

        pend = None  # quad whose tail is deferred one iteration (pipe mode)
        expfirst = os.environ.get("K_EXPFIRST", "0") == "1"
        for t in range(NT):
            sec, qi = divmod(t, NQ)
            j = QORDER[qi]
            ep = epp.tile([128, 8 * SF], bf16, tag="ep", name="ep")
            logit_quad(j, sec, ep)
            if expfirst:
                for h in range(nh):
                    nc.scalar.activation(
                        ep[:, h * HF : (h + 1) * HF],
                        ep[:, h * HF : (h + 1) * HF], EXP,
                    )
            if t + 1 < NT:
                vs2_next = convs(t + 1)  # PE prefetch: next quad's convs
            if not expfirst:
                for h in range(nh):
                    nc.scalar.activation(
                        ep[:, h * HF : (h + 1) * HF],
                        ep[:, h * HF : (h + 1) * HF], EXP,
                    )
            if cfg["pipe"]:
                if pend is not None:
                    tail(*pend)
                pend = (ep, vs2, sec, qi)
            else:
                tail(ep, vs2, sec, qi)
            if qi == 1 and sec + 1 < NSEC:
                qk_phase(sec + 1)  # overlap next section's q/kcv convs
            if t + 1 < NT:
                vs2 = vs2_next
        if pend is not None:
            tail(*pend)


def _build(cfg):
    key = tuple(sorted(cfg.items()))
    if key in _CACHE:
        return _CACHE[key]
    import concourse.tile as tile
    from concourse import bacc, mybir

    nc = bacc.Bacc(
        "TRN2", target_bir_lowering=False, debug=False, num_devices=NCORES
    )
    f32 = mybir.dt.float32
    bf16 = mybir.dt.bfloat16
    aps = {}
    aps["slab"] = nc.dram_tensor("slab", [6, SLAB_FREE], bf16,
                                 kind="ExternalInput").ap()
    aps["wts"] = nc.dram_tensor("wts", [6, 18 * 128], bf16,
                                kind="ExternalInput").ap()
    aps["ident"] = nc.dram_tensor("ident", [128, 128], bf16,
                                  kind="ExternalInput").ap()
    aps["out"] = nc.dram_tensor("out", [OC, SH_OUT_ROWS, W], f32,
                                kind="ExternalOutput").ap()

    with tile.TileContext(nc) as tc:
        _emit(nc, tc, aps, cfg)
    nc.compile()
    _CACHE[key] = nc
    return nc


def _host_prep(inputs, cfg):
    import ml_dtypes

    x = np.asarray(inputs["x"], np.float32)
    key_w = np.asarray(inputs["key_w"], np.float32)
    query_w = np.asarray(inputs["query_w"], np.float32)
    value_w = np.asarray(inputs["value_w"], np.float32)
    emb_a = np.asarray(inputs["emb_a"], np.float32)
    emb_b = np.asarray(inputs["emb_b"], np.float32)
    emb_mix = np.asarray(inputs["emb_mix"], np.float32)

    # emb softmax over m, then effective per-offset value matrices W_k [16,64,3]
    la = emb_mix @ emb_a  # (M, KS)
    lb = emb_mix @ emb_b  # (M, KS)
    eloG = (la[:, :, None] + lb[:, None, :]).reshape(M, KS * KS).astype(np.float64)
    eloG -= eloG.max(axis=0, keepdims=True)
    emb = np.exp(eloG)
    emb /= emb.sum(axis=0, keepdims=True)  # (M, 16)
    wk = np.einsum("mk,moc->koc", emb.astype(np.float32), value_w)  # (16,64,3)

    # stacked block-diagonal weights [6, 18*128]
    wts = np.zeros((6, 18 * 128), np.float32)
    mats = [query_w, key_w] + [wk[k] for k in range(KS * KS)]
    for s, mat in enumerate(mats):
        wts[0:3, 128 * s : 128 * s + 64] = mat.T
        wts[3:6, 128 * s + 64 : 128 * (s + 1)] = mat.T

    ident = np.eye(128, dtype=np.float32)

    # padded input, doubled slabs
    xp = np.zeros((B, IC, H + 2 * PAD, W + 2 * PAD), np.float32)
    xp[:, :, PAD : PAD + H, PAD : PAD + W] = x

    wtsb = wts.astype(ml_dtypes.bfloat16)
    identb = ident.astype(ml_dtypes.bfloat16)
    in_maps = []
    for c in range(NCORES):
        b, hh = c // 2, c % 2
        slab = np.zeros((6, SLAB_FREE), np.float32)
        r0 = hh * SH_OUT_ROWS
        slab[0:3, : SLAB_ROWS * W132] = xp[
            b, :, r0 : r0 + SLAB_ROWS, :
        ].reshape(IC, -1)
        slab[3:6, : SLAB_ROWS * W132] = xp[
            b, :, r0 + BLK : r0 + BLK + SLAB_ROWS, :
        ].reshape(IC, -1)
        in_maps.append({
            "slab": slab.astype(ml_dtypes.bfloat16),
            "wts": wtsb,
            "ident": identb,
        })
    return in_maps


def _ensure_ntff_hook():
    """The agent image's antenv lacks axon_hooks, so boot() could not register
    the NTFF profile hook. Inject the registry module and register the
    ctypes-based hook so run_bass_kernel_spmd(trace=True) can profile."""
    import types

    try:
        import antenv
    except ImportError:
        return
    if "antenv.axon_hooks" in sys.modules:
        return
    try:
        from trn_agent_boot.trn_boot import _ntff_profile_via_ctypes

        hook = _ntff_profile_via_ctypes("/opt/axon/libaxon_pjrt.so")
    except Exception:
        hook = None
    mod = types.ModuleType("antenv.axon_hooks")
    mod._hook = hook
    mod.set_axon_ntff_profile_hook = lambda h: setattr(mod, "_hook", h)
    mod.get_axon_ntff_profile_hook = lambda: mod._hook
    sys.modules["antenv.axon_hooks"] = mod
    antenv.axon_hooks = mod


def kernel(**inputs):
    global LAST_RESULT
    cfg = dict(CFG)
    in_maps = _host_prep(inputs, cfg)
    nc = _build(cfg)

    from concourse.bass_utils import run_bass_kernel_spmd

    trace = os.environ.get("KERNEL_TRACE", "0") == "1"
    if trace:
        _ensure_ntff_hook()
    res = run_bass_kernel_spmd(
        nc, in_maps, core_ids=list(range(NCORES)), trace=trace
    )
    LAST_RESULT = res

    out = np.empty((B, OC, H, W), np.float32)
    for c in range(NCORES):
        b, hh = c // 2, c % 2
        out[b, :, hh * SH_OUT_ROWS : (hh + 1) * SH_OUT_ROWS, :] = res.results[c]["out"]
    return out

